# revision 5
# baseline (speedup 1.0000x reference)
"""Multi-head attention (B=2, S=2048, D=1024, H=16) on 8 TRN2 NeuronCores.

Sharding: core c handles batch b = c//4 and heads [4*(c%4), 4*(c%4)+4) —
tensor-parallel over heads x data-parallel over batch.  Each core computes a
partial output projection (its heads' contribution); the host sums the 4
partials per batch and adds b_out.

The kernel is paced by the Scalar (ACT) engine's exp stream (16.8M score
elements per core, 1 elem/lane/cycle @1.2GHz ~= 16us per 512-sq block); the
layout keeps ACT saturated and fits everything else into PE/DVE/DMA slack:
  - input DMA is chunked (yT per 512-seq block) so the qk projection starts
    ~3.5us in instead of after the full 6MB load.
  - only the q/k projection for head pair 0 runs before the first attention
    block; pair 1's projection and the v projection run inside the first
    blocks' PE slack (ACT-bound), as do the output projections later.
  - score PSUM is triple-buffered per head so the exp ACTs never wait on the
    PE; exp activations stream back-to-back at (1024+176)/1.2 ~= 1us.
  - attn@v accumulates v_aug.T @ expT over sk chunks in reversed order (one
    ACT wait, then back-to-back); the appended ones-column yields the softmax
    denominator as PSUM row 64.  No max-subtraction: scores*scale ~N(0,1).
  - both heads' denominator rows are staged to one SBUF tile (DVE copy +
    partition-shift DMA) so one [2,512] DVE reciprocal serves the pair.
  - normalization (DRAM-bounce stride-0 broadcast of the reciprocal, DVE
    multiply) and the output projection for a 512-block are deferred one
    block so they ride in later blocks' slack; PSUM->SBUF staging of the
    output projection runs on DVE, keeping ACT exp-only.
"""
import sys

sys.path.insert(0, "/opt/trn_rl_repo")

import numpy as np

B, S, D = 2, 2048, 1024
H, Hd = 16, 64
P = 128
NKC = D // P      # 8 contraction chunks for the projections
NSC = S // P      # 16 sequence chunks of 128
SQB = 512         # sq block size
NSQB = S // SQB   # 4

_CACHE = {}


def _build_nc():
    import concourse.mybir as mybir
    import concourse.tile as tile
    from concourse import bacc

    f32 = mybir.dt.float32
    bf16 = mybir.dt.bfloat16
    AF = mybir.ActivationFunctionType

    nc = bacc.Bacc(None, target_bir_lowering=False, debug=False)

    yT_d = nc.dram_tensor("yT", [D, S], bf16, kind="ExternalInput")[:]
    Wqk_d = nc.dram_tensor("WqkT", [D, 512], bf16, kind="ExternalInput")[:]
    bqk_d = nc.dram_tensor("bqk", [P, 4], f32, kind="ExternalInput")[:]
    Wv_d = nc.dram_tensor("WvT", [D, 256], bf16, kind="ExternalInput")[:]
    Wout_d = nc.dram_tensor("WoutT", [256, D], bf16, kind="ExternalInput")[:]
    out_d = nc.dram_tensor("out", [S, D], f32, kind="ExternalOutput")[:]

    with tile.TileContext(nc) as tc:
        with (
            tc.tile_pool(name="persist", bufs=1) as persist,
            tc.tile_pool(name="p2e", bufs=4) as p2e,
            tc.tile_pool(name="p2s", bufs=2) as p2s,
            tc.tile_pool(name="scoreps", bufs=3, space="PSUM") as scoreps,
            tc.tile_pool(name="auxps", bufs=2, space="PSUM") as auxps,
            tc.tile_pool(name="p2dram", bufs=2, space="DRAM") as p2dram,
        ):
            # ---- persistent SBUF tensors ----
            Wqk_sb = persist.tile([P, NKC, 512], bf16)
            Wv_sb = persist.tile([P, NKC, 256], bf16)
            yT_sb = persist.tile([P, NKC, S], bf16)
            Wout_sb = persist.tile([P, 2, D], bf16)
            bqk_sb = persist.tile([P, 4], f32)
            qkT_sb = persist.tile([P, 4, S], bf16)
            v_sb = persist.tile([P, NSC, 4, 65], bf16)
            vT_sb = persist.tile([P, 2, S], bf16)
            ones_f32 = persist.tile([P, 1], f32)

            # ---- input DMA, ordered so compute can start early ----
            Wqkr = Wqk_d.rearrange("(kc p) e -> p kc e", p=P)
            yTr = yT_d.rearrange("(kc p) s -> p kc s", p=P)
            nc.sync.dma_start(bqk_sb[:], bqk_d)
            for m in range(2):
                nc.sync.dma_start(Wqk_sb[:, :, m * P:(m + 1) * P],
                                  Wqkr[:, :, m * P:(m + 1) * P])
            for sb in range(4):
                sq = slice(sb * 512, (sb + 1) * 512)
                for kc in range(NKC):
                    nc.sync.dma_start(yT_sb[:, kc, sq], yTr[:, kc, sq])
            nc.sync.dma_start(Wv_sb[:], Wv_d.rearrange("(kc p) e -> p kc e",
                                                       p=P))
            for m in range(2, 4):
                nc.sync.dma_start(Wqk_sb[:, :, m * P:(m + 1) * P],
                                  Wqkr[:, :, m * P:(m + 1) * P])
            nc.sync.dma_start(Wout_sb[:], Wout_d.rearrange(
                "(kc p) e -> p kc e", p=P))

            nc.any.memset(ones_f32[:], 1.0)
            nc.vector.tensor_copy(
                v_sb[:, :, :, 64:65],
                ones_f32.unsqueeze(1).unsqueeze(1).to_broadcast(
                    (P, NSC, 4, 1)))

            # ---- building blocks ----
            def qkproj(m, sb):
                sq = slice(sb * 512, (sb + 1) * 512)
                ps = auxps.tile([P, 512], f32, tag="aux", name="ps")
                for kc in range(NKC):
                    nc.tensor.matmul(
                        ps[:], Wqk_sb[:, kc, m * P:(m + 1) * P],
                        yT_sb[:, kc, sq],
                        start=(kc == 0), stop=(kc == NKC - 1))
                nc.vector.tensor_add(
                    qkT_sb[:, m, sq], ps[:],
                    bqk_sb[:, m:m + 1].to_broadcast((P, 512)))

            def vproj(sc):
                psv = auxps.tile([P, 256], f32, tag="aux", name="psv")
                for kc in range(NKC):
                    nc.tensor.matmul(
                        psv[:], yT_sb[:, kc, sc * P:(sc + 1) * P],
                        Wv_sb[:, kc, :],
                        start=(kc == 0), stop=(kc == NKC - 1))
                nc.vector.tensor_copy(
                    v_sb[:, sc, :, 0:64],
                    psv.rearrange("p (i d) -> p i d", i=4))

            def attnv_block(p, sqb, ex):
                """attn@v for a completed score/exp block; stages values to
                SBUF, batches both heads' denominators into one reciprocal,
                and bounces the reciprocals through DRAM for broadcast."""
                den_sb = p2s.tile([P, 2, SQB], f32, tag="den", name="den_sb")
                vals2 = []
                for sub in range(2):
                    i = 2 * p + sub
                    psv2 = auxps.tile([P, SQB], f32, tag="aux", name="psv2")
                    # reversed order: only the first matmul waits on ACT
                    # (all exps of this tile done); the rest issue
                    # back-to-back so LDWEIGHTS hides
                    for mk in range(NSC - 1, -1, -1):
                        nc.tensor.matmul(
                            psv2[0:65, :], v_sb[:, mk, i, :],
                            ex[sub][:, mk, :],
                            start=(mk == NSC - 1), stop=(mk == 0))
                    vals = p2s.tile([64, SQB], f32, tag="vals", name="vals",
                                    bufs=4)
                    nc.vector.tensor_copy(vals[:], psv2[0:64, :])
                    nc.vector.tensor_copy(den_sb[64:65, sub, :],
                                          psv2[64:65, :])
                    vals2.append(vals)
                # partition-shift the two denominator rows onto partitions
                # 0/1 of one tile so a single DVE reciprocal covers both
                rstage = p2s.tile([2, SQB], f32, tag="rstage", name="rstage")
                for sub in range(2):
                    nc.sync.dma_start(rstage[sub:sub + 1, :],
                                      den_sb[64:65, sub, :])
                rrec = p2s.tile([2, SQB], f32, tag="rrec", name="rrec")
                nc.vector.reciprocal(rrec[:], rstage[:])
                rdram = p2dram.tile([2, SQB], f32, name="rdram")
                nc.sync.dma_start(rdram[:], rrec[:])
                return (p, sqb, vals2, rdram)

            def normalize(p, sqb, vals2, rdram):
                sq = slice(sqb * SQB, (sqb + 1) * SQB)
                for sub in range(2):
                    # broadcast recip row across 64 partitions by re-reading
                    # the DRAM copy with a stride-0 partition dimension
                    # (v-bias is folded into the output on the host)
                    rbs = p2s.tile([64, SQB], f32, tag="rbs", name="rbs")
                    nc.sync.dma_start(
                        rbs[:], rdram[sub:sub + 1, :].to_broadcast(
                            (64, SQB)))
                    vtmp = p2s.tile([64, SQB], bf16, tag="vtmp", name="vtmp")
                    nc.vector.tensor_mul(vtmp[:], vals2[sub][:], rbs[:])
                    nc.sync.dma_start(
                        vT_sb[sub * 64:(sub + 1) * 64, p, sq], vtmp[:])

            def outproj(sqb):
                for sc in range(sqb * 4, sqb * 4 + 4):
                    for nb in range(2):
                        pso = auxps.tile([P, 512], f32, tag="aux",
                                         name="pso")
                        for kc in range(2):
                            nc.tensor.matmul(
                                pso[:], vT_sb[:, kc, sc * P:(sc + 1) * P],
                                Wout_sb[:, kc, nb * 512:(nb + 1) * 512],
                                start=(kc == 0), stop=(kc == 1))
                        ost = p2s.tile([P, 512], f32, tag="ost", name="ost",
                                       bufs=3)
                        nc.vector.tensor_copy(ost[:], pso[:])
                        nc.sync.dma_start(
                            out_d[sc * P:(sc + 1) * P,
                                  nb * 512:(nb + 1) * 512], ost[:])

            # ---- lead-in: qk projection for head pair 0 only ----
            for m in range(2):
                for sb in range(4):
                    qkproj(m, sb)

            # ---- attention blocks ----
            # Block order interleaves the two head pairs so the output
            # projections spread across the second half instead of piling
            # into the tail; pair 1's scores (position 3) leave blocks 1-2
            # of PE slack for its qk projection.
            blocks = [(0, 0), (0, 1), (0, 2), (1, 0),
                      (0, 3), (1, 1), (1, 2), (1, 3)]
            fillers = {
                0: [lambda sc=sc: vproj(sc) for sc in range(NSC)],
                1: [lambda sb=sb: qkproj(2, sb) for sb in range(4)],
                2: [lambda sb=sb: qkproj(3, sb) for sb in range(4)],
            }
            prev = None
            pending = []
            normed = set()

            def flush_one():
                p, sqb, vals2, rdram = pending.pop(0)
                normalize(p, sqb, vals2, rdram)
                normed.add((p, sqb))
                if (0, sqb) in normed and (1, sqb) in normed:
                    outproj(sqb)

            for bi, (p, sqb) in enumerate(blocks):
                sq = slice(sqb * SQB, (sqb + 1) * SQB)
                exa = p2e.tile([P, NSC, SQB], bf16, tag="exp")
                exb = p2e.tile([P, NSC, SQB], bf16, tag="exp")
                ex = (exa, exb)
                # two sk-chunks share one 2-bank PSUM tile; a single exp
                # activation covers both (halves ACT op count).  The h0/h64
                # matmuls of the two heads are adjacent so they run
                # concurrently in distinct PE row-groups.
                for mj in range(NSC // 2):
                    pss = [
                        scoreps.tile([P, 2, SQB], f32, tag="score",
                                     name="pss")
                        for _ in range(2)]
                    for half in range(2):
                        mk = 2 * mj + half
                        for sub in range(2):
                            prt = slice(sub * 64, (sub + 1) * 64)
                            nc.tensor.matmul(
                                pss[sub][:, half, :],
                                qkT_sb[prt, 2 * p + 1, mk * P:(mk + 1) * P],
                                qkT_sb[prt, 2 * p, sq])
                    for sub in range(2):
                        nc.scalar.activation(
                            ex[sub][:, 2 * mj:2 * mj + 2, :],
                            pss[sub][:], AF.Exp, scale=0.125)
                # previous block's attn@v and the projection fillers ride
                # in this block's PE slack while ACT streams the exps
                if prev is not None:
                    pending.append(attnv_block(*prev))
                for f in fillers.get(bi, []):
                    f()
                while len(pending) > 1:
                    flush_one()
                prev = (p, sqb, ex)
            pending.append(attnv_block(*prev))
            while pending:
                flush_one()

    nc.compile()
    return nc


def _get_nc():
    if "nc" not in _CACHE:
        _CACHE["nc"] = _build_nc()
    return _CACHE["nc"]


def _host_prep(y, W_qkv, b_qkv, W_out, c):
    b = c // 4
    q = c % 4
    hs = [4 * q + i for i in range(4)]

    def Wrow(h, part):
        return W_qkv[h * 192 + part * 64: h * 192 + (part + 1) * 64]

    def brow(h, part):
        return b_qkv[h * 192 + part * 64: h * 192 + (part + 1) * 64]

    qk_rows = np.concatenate([
        Wrow(hs[0], 0), Wrow(hs[1], 0), Wrow(hs[0], 1), Wrow(hs[1], 1),
        Wrow(hs[2], 0), Wrow(hs[3], 0), Wrow(hs[2], 1), Wrow(hs[3], 1)],
        axis=0)
    bqk_flat = np.concatenate([
        brow(hs[0], 0), brow(hs[1], 0), brow(hs[0], 1), brow(hs[1], 1),
        brow(hs[2], 0), brow(hs[3], 0), brow(hs[2], 1), brow(hs[3], 1)],
        axis=0)
    import ml_dtypes

    bf = ml_dtypes.bfloat16
    WqkT = np.ascontiguousarray(qk_rows.T.astype(bf))        # [1024, 512]
    bqk = np.ascontiguousarray(bqk_flat.reshape(4, P).T)     # [128, 4]
    WvT = np.ascontiguousarray(
        np.concatenate([Wrow(h, 2) for h in hs], axis=0).T.astype(bf))
    dsl = np.concatenate([np.arange(h * 64, (h + 1) * 64) for h in hs])
    WoutT = np.ascontiguousarray(W_out[:, dsl].T.astype(bf))  # [256, 1024]
    yT = np.ascontiguousarray(y[b].T.astype(bf))             # [1024, 2048]
    return {"yT": yT, "WqkT": WqkT, "bqk": bqk, "WvT": WvT,
            "WoutT": WoutT}


def _gather(results, b_qkv, W_out, b_out):
    parts = [results[c]["out"] for c in range(8)]
    # v-bias commutes through the output projection: fold it host-side
    bv_full = b_qkv.reshape(16, 3, 64)[:, 2, :].reshape(1024)
    bias = b_out + bv_full @ W_out.T
    return np.stack([
        parts[0] + parts[1] + parts[2] + parts[3] + bias,
        parts[4] + parts[5] + parts[6] + parts[7] + bias,
    ]).astype(np.float32)


def kernel(y, W_qkv, b_qkv, W_out, b_out):
    from concourse.bass_utils import run_bass_kernel_spmd

    y = np.ascontiguousarray(np.asarray(y, dtype=np.float32))
    W_qkv = np.ascontiguousarray(np.asarray(W_qkv, dtype=np.float32))
    b_qkv = np.ascontiguousarray(np.asarray(b_qkv, dtype=np.float32))
    W_out = np.ascontiguousarray(np.asarray(W_out, dtype=np.float32))
    b_out = np.asarray(b_out, dtype=np.float32)

    nc = _get_nc()
    in_maps = [_host_prep(y, W_qkv, b_qkv, W_out, c) for c in range(8)]
    res = run_bass_kernel_spmd(nc, in_maps, core_ids=list(range(8)))
    return _gather(res.results, b_qkv, W_out, b_out)
